# revision 5
# baseline (speedup 1.0000x reference)
"""GNN message passing (scatter-add of gathered node features) on 8 TRN2 NeuronCores.

v3 strategy (dest-sharded, on-chip scatter via one-hot matmul, 64-row tiles):
  - Outputs node-sharded: core k owns dest rows [k*12500, (k+1)*12500),
    covered by 196 "tiles" of 64 rows (psum-packed two tiles per 128
    partitions: even tiles at partitions 0-63, odd at 64-127).
  - Token stream: 4 segment sweeps (sources segmented so gather indices fit
    int16); within a sweep, one "cell" per tile. Cells padded to the max
    count over the 8 cores so all cores share ONE program layout (SPMD).
  - Gather: big dma_gather calls (CAP=48 chunks x 128 tokens) pull x[col]
    rows (256B) from HBM into an SBUF fp32 ring; ACT casts fp32->bf16.
  - One-hot builds: iota==sidx compares (8 pairs / instruction), split
    between DVE and GpSimd (GSHARE) with separate completion semaphores.
  - PE accumulates psum_cell += onehot^T @ msg per (chunk, tile) pair;
    lhsT is [128, 64] so each matmul writes a 64-partition psum slice.
    Psum slot for tile t: partitions (t%2)*64, bank (t//2)%8, offset
    ((t//16)%8)*64 floats -> 128 distinct slots, reuse distance 128 cells.
  - DVE retires 4 consecutive cells per op ([128p, 2 banks, 64f] add into
    out_sb); SP (HWDGE) streams final out_sb to HBM per tile-quarter.
"""

import numpy as np

N_NODES = 100000
N_EDGES = 1250000
D = 64
NCORES = 8
SHARD = N_NODES // NCORES      # 12500 dest rows per core
TILE = 64
NTILES = -(-SHARD // TILE)     # 196
NROWS_PAD = NTILES * TILE      # 12544
NSEG = 4
SEGSZ = N_NODES // NSEG        # 25000
CPS = 200                      # cells per sweep: 196 real tiles + 4 pad cells
CAP = 48        # max chunks (of 128 tokens) per dma_gather call
RINGC = 192     # message ring size in chunks (fp32 and bf16 rings)
RO = 96         # one-hot ring size in pairs
BATCH = 8       # pairs per one-hot instruction
NSLOT = 128     # distinct psum slots (2 halves x 8 banks x 8 offsets)
BANK = 512      # fp32 elements per psum bank
RB = 4          # cells retired per DVE retire op
NOUT = CPS // 2  # out_sb col blocks of 64 (98 real + 2 trash)
GSHARE = 0      # every GSHARE-th one-hot batch goes to GpSimd (0 = none)
                # (GpSimd/Pool rejects TENSOR_TENSOR in the V3 ISA check)


class Layout:
    pass


def build_layout(counts):
    """counts: [NCORES, NTILES, NSEG] per-cell token counts.
    Returns the SPMD-shared stream layout + schedule."""
    ly = Layout()
    L = counts.max(axis=0)                      # [NTILES, NSEG] shared cell len
    cell_off = np.zeros((NTILES, NSEG), np.int64)
    sweeps = []                                 # (seg, tok0, ntok)
    off = 0
    for s in range(NSEG):
        tok0 = off
        for t in range(NTILES):
            cell_off[t, s] = off
            off += int(L[t, s])
        off += (-(off - tok0)) % 128            # pad sweep to chunk boundary
        sweeps.append((s, tok0, off - tok0))
    T = off
    nchunks = T // 128
    sweep_last_chunk = [(tok0 + ntok) // 128 - 1 for s, tok0, ntok in sweeps]

    # ---- gather calls: split each sweep into <=CAP chunk spans ----
    calls = []                                  # (c0, nch, seg)
    for s, tok0, ntok in sweeps:
        nch = ntok // 128
        c0 = tok0 // 128
        for a in range(0, nch, CAP):
            calls.append((c0 + a, min(CAP, nch - a), s))

    # message-ring slot per chunk; never wrap inside one call
    chunk_slot = np.full(nchunks, -1, np.int64)
    call_slot0 = []
    wait_cast_call = []          # per call: latest earlier call sharing slots
    last_writer = np.full(RINGC, -1, np.int64)
    cur = 0
    for j, (c0, nch, s) in enumerate(calls):
        if cur % RINGC + nch > RINGC:
            cur += (-cur) % RINGC
        s0 = cur % RINGC
        wait_cast_call.append(int(last_writer[s0:s0 + nch].max()))
        chunk_slot[c0:c0 + nch] = np.arange(s0, s0 + nch)
        last_writer[s0:s0 + nch] = j
        call_slot0.append(s0)
        cur += nch
    call_of_chunk = np.full(nchunks, -1, np.int64)
    for j, (c0, nch, s) in enumerate(calls):
        call_of_chunk[c0:c0 + nch] = j

    ly.sweep_last_chunk = sweep_last_chunk
    ly.sweep_of_chunk = np.zeros(nchunks, np.int64)
    for s, tok0, ntok in sweeps:
        ly.sweep_of_chunk[tok0 // 128:(tok0 + ntok) // 128] = s

    ly.L, ly.cell_off, ly.sweeps, ly.T, ly.nchunks = L, cell_off, sweeps, T, nchunks
    ly.calls, ly.chunk_slot, ly.call_slot0 = calls, chunk_slot, call_slot0
    ly.wait_cast_call, ly.call_of_chunk = wait_cast_call, call_of_chunk
    return ly


def host_prep(x, edge_index):
    x = np.ascontiguousarray(np.asarray(x, dtype=np.float32))
    row = np.asarray(edge_index[0], dtype=np.int64)
    col = np.asarray(edge_index[1], dtype=np.int64)
    core = row // SHARD
    r_loc = row - core * SHARD
    tile = r_loc // TILE
    seg = col // SEGSZ

    counts = np.zeros((NCORES, NTILES, NSEG), np.int64)
    np.add.at(counts, (core, tile, seg), 1)
    ly = build_layout(counts)

    # cell pieces: shared cell span split at gather-call boundaries
    call_start_tok = {j: c0 * 128 for j, (c0, nch, s) in enumerate(ly.calls)}
    call_end_tok = {j: (c0 + nch) * 128 for j, (c0, nch, s) in enumerate(ly.calls)}
    cell_pieces = {}                     # (t, s) -> [(call j, piece_len), ...]
    for s in range(NSEG):
        for t in range(NTILES):
            a = int(ly.cell_off[t, s])
            b = a + int(ly.L[t, s])
            pieces = []
            p = a
            while p < b:
                j = int(ly.call_of_chunk[p // 128])
                e = min(b, call_end_tok[j])
                pieces.append((j, e - p))
                p = e
            cell_pieces[(t, s)] = pieces

    # per-core compacted placement + union pair incidence
    pair_set = set()
    gidx_all = np.full((NCORES, ly.T), -1, np.int16)
    sraw_all = np.full((NCORES, ly.T), -1, np.int16)
    per_vals = []
    for k in range(NCORES):
        m = core == k
        rl, c_, t_, s_ = r_loc[m], col[m], tile[m], seg[m]
        key = s_ * NTILES + t_
        order = np.argsort(key, kind="stable")
        rl, c_, t_, s_ = rl[order], c_[order], t_[order], s_[order]
        gval = (c_ - s_ * SEGSZ).astype(np.int16)
        ks = (s_ * NTILES + t_)
        starts = np.r_[0, np.nonzero(np.diff(ks))[0] + 1]
        cnts = np.diff(np.r_[starts, len(ks)])
        per_vals.append((rl, gval, t_, s_, starts, cnts, ks))

    gcnt_all = np.zeros((NCORES, len(ly.calls)), np.int32)
    for k in range(NCORES):
        rl, gval, t_, s_, starts, cnts, ks = per_vals[k]
        cur = dict(call_start_tok)       # per-call compaction cursor
        idx_of_group = {int(ks[st]): (int(st), int(cn))
                        for st, cn in zip(starts, cnts)}
        for s in range(NSEG):
            for t in range(NTILES):
                g = idx_of_group.get(s * NTILES + t)
                if g is None:
                    continue
                st, cn = g
                off = 0
                for (j, plen) in cell_pieces[(t, s)]:
                    take = min(cn - off, plen)
                    if take <= 0:
                        break
                    p0 = cur[j]
                    cur[j] = p0 + take
                    gidx_all[k, p0:p0 + take] = gval[st + off:st + off + take]
                    sraw_all[k, p0:p0 + take] = rl[st + off:st + off + take]
                    ca, cb = p0 // 128, (p0 + take - 1) // 128
                    for c in range(ca, cb + 1):
                        pair_set.add((c, t))
                    off += take
        for j in range(len(ly.calls)):
            gcnt_all[k, j] = cur[j] - call_start_tok[j]

    # insurance pairs: empty cells (all cores) and pad cells
    covered = {}
    for c, t in pair_set:
        cell = int(ly.sweep_of_chunk[c]) * CPS + t
        covered[cell] = True
    for s in range(NSEG):
        for t in range(NTILES):
            if (s * CPS + t) not in covered:
                c = min(int(ly.cell_off[t, s]) // 128, ly.sweep_last_chunk[s])
                pair_set.add((c, t))
        for t in range(NTILES, CPS):
            pair_set.add((ly.sweep_last_chunk[s], t))

    pairs = sorted(pair_set)
    npairs = len(pairs)
    npairs_pad = npairs + (-npairs) % BATCH
    pair_chunks = np.array([c for c, t in pairs], np.int64)
    pair_tiles = np.array([t for c, t in pairs], np.int64)
    pair_cell = ly.sweep_of_chunk[pair_chunks] * CPS + pair_tiles
    ncells = NSEG * CPS
    first_pair = np.full(ncells, -1, np.int64)
    last_pair = np.full(ncells, -1, np.int64)
    for p in range(npairs):
        i = int(pair_cell[p])
        if first_pair[i] < 0:
            first_pair[i] = p
        last_pair[i] = p
    assert (first_pair >= 0).all(), "every cell needs a pair"
    # psum slot reuse safety: cell c and its previous same-slot cell
    prev_same_slot = np.full(ncells, -1, np.int64)
    last_of_slot = {}
    for c in range(ncells):
        key = (c % CPS) % NSLOT
        if key in last_of_slot:
            prev_same_slot[c] = last_of_slot[key]
        last_of_slot[key] = c
    for c in range(ncells):
        pc = prev_same_slot[c]
        if pc >= 0:
            assert last_pair[pc] < first_pair[c], (c, pc)

    last_pair_of_call = []
    for j, (c0, nch, s) in enumerate(ly.calls):
        last_pair_of_call.append(int(np.searchsorted(pair_chunks, c0 + nch) - 1))

    nrb = ncells // RB
    retire_after_pair = [int(last_pair[RB * b:RB * b + RB].max()) for b in range(nrb)]

    # one-hot batch -> engine assignment (GpSimd takes every GSHARE-th batch)
    nbatch = npairs_pad // BATCH
    batch_on_g = [bool(GSHARE) and (b % GSHARE == GSHARE - 1) for b in range(nbatch)]
    # for PE waits: per batch, how many dve/gps batches completed once batch b done
    dve_done = np.zeros(nbatch, np.int64)
    gps_done = np.zeros(nbatch, np.int64)
    nd = ng = 0
    for b in range(nbatch):
        if batch_on_g[b]:
            ng += 1
        else:
            nd += 1
        dve_done[b] = nd
        gps_done[b] = ng

    ly.pairs, ly.npairs, ly.npairs_pad = pairs, npairs, npairs_pad
    ly.pair_chunks, ly.pair_tiles, ly.pair_cell = pair_chunks, pair_tiles, pair_cell
    ly.first_pair, ly.last_pair, ly.ncells = first_pair, last_pair, ncells
    ly.prev_same_slot = prev_same_slot
    ly.last_pair_of_call = last_pair_of_call
    ly.nrb, ly.retire_after_pair = nrb, retire_after_pair
    ly.nbatch, ly.batch_on_g = nbatch, batch_on_g
    ly.dve_done, ly.gps_done = dve_done, gps_done

    per_core = []
    for k in range(NCORES):
        gw = np.tile(gidx_all[k].reshape(-1, 16).T, (8, 1)).copy()
        S = sraw_all[k].reshape(ly.nchunks, 128)
        sp = np.full((128, npairs_pad), -1, np.int16)
        sp[:, :npairs] = (
            S[pair_chunks].astype(np.int32) - (pair_tiles * TILE)[:, None]
        ).T.astype(np.int16)
        per_core.append({"x": x, "gidx": gw, "sidx": sp,
                         "gcnt": gcnt_all[k:k + 1, :].copy()})
    return per_core, ly


def simulate_numpy(per_core, ly):
    """Numpy model of the device program (validates host layout logic)."""
    outs = []
    iota = np.arange(TILE, dtype=np.int32)
    for k in range(NCORES):
        x = per_core[k]["x"]
        gw = per_core[k]["gidx"]
        sp = per_core[k]["sidx"]
        gidx = gw[:16, :].T.reshape(-1).astype(np.int64)
        segs = np.zeros(ly.T, np.int64)
        for s, tok0, ntok in ly.sweeps:
            segs[tok0:tok0 + ntok] = s
        mb = x[np.clip(gidx, 0, None) + segs * SEGSZ].astype(np.float32)  # [T, 64]
        acc = np.zeros((CPS, TILE, D), np.float32)
        for p, (c, t) in enumerate(ly.pairs):
            oh = (sp[:, p:p + 1].astype(np.int32) == iota[None, :])
            acc[t] += oh.T.astype(np.float32) @ mb[c * 128:(c + 1) * 128]
        outs.append(acc[:NTILES].reshape(NROWS_PAD, D)[:SHARD])
    return np.concatenate(outs, axis=0)


def build_bass(ly):
    import concourse.bacc as bacc
    import concourse.mybir as mybir

    nc = bacc.Bacc(None, target_bir_lowering=False, debug=False, num_swdge_queues=4)
    x = nc.dram_tensor("x", [N_NODES, D], mybir.dt.float32, kind="ExternalInput")
    gidx = nc.dram_tensor("gidx", [128, ly.T // 16], mybir.dt.int16, kind="ExternalInput")
    sidx = nc.dram_tensor("sidx", [128, ly.npairs_pad], mybir.dt.int16, kind="ExternalInput")
    outd = nc.dram_tensor("out", [128, (NTILES // 2) * D], mybir.dt.float32, kind="ExternalOutput")
    gcnt = nc.dram_tensor("gcnt", [1, len(ly.calls)], mybir.dt.int32, kind="ExternalInput")

    class W:
        """per-engine monotonic semaphore wait dedup"""

        def __init__(self, eng):
            self.eng = eng
            self.last = {}

        def wait(self, sem, v):
            if v > 0 and self.last.get(id(sem), 0) < v:
                self.eng.wait_ge(sem, v)
                self.last[id(sem)] = v

    import contextlib
    with contextlib.ExitStack() as stack:
        ec = stack.enter_context
        gi_sb = ec(nc.sbuf_tensor([128, ly.T // 16], mybir.dt.int16))
        gc_sb = ec(nc.sbuf_tensor([1, len(ly.calls)], mybir.dt.int32))
        sp_sb = ec(nc.sbuf_tensor([128, ly.npairs_pad], mybir.dt.int16))
        iota_sb = ec(nc.sbuf_tensor([128, BATCH * TILE], mybir.dt.int16))
        msgf = ec(nc.sbuf_tensor([128, RINGC * D], mybir.dt.float32))
        msgb = ec(nc.sbuf_tensor([128, RINGC * D], mybir.dt.bfloat16))
        oh_sb = ec(nc.sbuf_tensor([128, RO * TILE], mybir.dt.bfloat16))
        out_sb = ec(nc.sbuf_tensor([128, NOUT * D], mybir.dt.float32))
        psum = ec(nc.psum_tensor("acc", [128, 8 * BANK], mybir.dt.float32))
        lsem = ec(nc.semaphore("lsem"))
        NGS = 8
        gsem = [ec(nc.semaphore(f"gsem{i}")) for i in range(NGS)]
        c_sem = ec(nc.semaphore("c_sem"))
        ohd_sem = ec(nc.semaphore("ohd_sem"))
        ohg_sem = ec(nc.semaphore("ohg_sem"))
        mm_sem = ec(nc.semaphore("mm_sem"))
        rt_sem = ec(nc.semaphore("rt_sem"))
        o_sem = ec(nc.semaphore("o_sem"))

        def psum_cell_ap(t):
            """psum AP [64p, 64f] for tile t (any sweep)."""
            half = t % 2
            bank = (t // 2) % 8
            off = ((t // 16) % 8) * D
            return psum[half * 64:(half + 1) * 64,
                        bank * BANK + off:bank * BANK + off + D]

        def onehot_op(eng, b, w, sem):
            j0 = BATCH * b
            o0 = j0 % RO
            eng.tensor_tensor(
                out=oh_sb[:, o0 * TILE:(o0 + BATCH) * TILE].rearrange(
                    "p (j f) -> p j f", f=TILE),
                in0=iota_sb[:].rearrange("p (j f) -> p j f", f=TILE),
                in1=sp_sb[:, j0:j0 + BATCH][:, :, None].broadcast_to(
                    [128, BATCH, TILE]),
                op=mybir.AluOpType.is_equal,
            ).then_inc(sem, 1)

        with nc.Block() as block:

            @block.sync
            def _(sync):
                sync.dma_start(out=gi_sb[:], in_=gidx[:]).then_inc(lsem, 16)
                sync.dma_start(out=sp_sb[:], in_=sidx[:]).then_inc(lsem, 16)
                sync.dma_start(out=gc_sb[:], in_=gcnt[:]).then_inc(lsem, 16)
                sync.wait_ge(lsem, 48)

            @block.gpsimd
            def _(g):
                g.iota(
                    iota_sb[:].rearrange("p (j f) -> p j f", f=TILE),
                    [[0, BATCH], [1, TILE]],
                    base=0,
                    channel_multiplier=0,
                )

            @block.vector
            def _(vector):
                vector.memset(out_sb[:], 0)
                vector.memset(msgf[:], 0)

        with nc.Block() as block:

            @block.gpsimd
            def _(g):
                w = W(g)
                # merged schedule: gather calls + assigned one-hot batches.
                # one-hot batch b may be emitted once its oh-ring slot is free
                # (mm_sem) -- no data dependency on the gather stream.
                gb = [b for b in range(ly.nbatch) if ly.batch_on_g[b]]
                gi = 0
                with g.register("cnt") as creg:
                    for j, (c0, nch, s) in enumerate(ly.calls):
                        # emit gpsimd one-hot batches that are ready (ring
                        # slot free given matmuls already gated by this call)
                        while gi < len(gb) and BATCH * (gb[gi] + 1) - RO <= (
                                ly.last_pair_of_call[j - 1] + 1 if j > 0 else 0):
                            b = gb[gi]
                            w.wait(mm_sem, BATCH * (b + 1) - RO)
                            onehot_op(g, b, w, ohg_sem)
                            gi += 1
                        w.wait(c_sem, ly.wait_cast_call[j] + 1)
                        s0 = ly.call_slot0[j]
                        tok0 = c0 * 128
                        g.reg_load(creg, gc_sb[:1, j:j + 1])
                        g.dma_gather(
                            out_ap=msgf[:, s0 * D:(s0 + nch) * D].rearrange(
                                "p (k dd) -> p k dd", dd=D),
                            in_ap=x[s * SEGSZ:(s + 1) * SEGSZ, :],
                            idxs_ap=gi_sb[:, tok0 // 16:(tok0 + nch * 128) // 16],
                            num_idxs=nch * 128,
                            num_idxs_reg=creg,
                            elem_size=D,
                            queue_num=j % 4,
                            single_packet=False,
                        ).then_inc(gsem[j % NGS], 16)
                    while gi < len(gb):
                        b = gb[gi]
                        w.wait(mm_sem, BATCH * (b + 1) - RO)
                        onehot_op(g, b, w, ohg_sem)
                        gi += 1

            @block.scalar
            def _(act):
                w = W(act)
                for j, (c0, nch, s) in enumerate(ly.calls):
                    w.wait(gsem[j % NGS], 16 * (j // NGS + 1))
                    prev = ly.wait_cast_call[j]
                    if prev >= 0:
                        w.wait(mm_sem, ly.last_pair_of_call[prev] + 1)
                    s0 = ly.call_slot0[j]
                    act.copy(
                        out=msgb[:, s0 * D:(s0 + nch) * D],
                        in_=msgf[:, s0 * D:(s0 + nch) * D],
                    ).then_inc(c_sem, 1)

            @block.vector
            def _(vector):
                w = W(vector)
                rb = 0
                for b in range(ly.nbatch):
                    # retire cells whose pairs are fully covered by emitted
                    # one-hot batches (they can only retire after those mms)
                    while rb < ly.nrb and ly.retire_after_pair[rb] < BATCH * b:
                        w.wait(mm_sem, ly.retire_after_pair[rb] + 1)
                        cell0 = RB * rb
                        t0 = cell0 % CPS
                        bank = (t0 // 2) % 8
                        off = ((t0 // 16) % 8) * D
                        vector.tensor_tensor(
                            out=out_sb[:, (t0 // 2) * D:(t0 // 2 + 2) * D].rearrange(
                                "p (j f) -> p j f", f=D),
                            in0=out_sb[:, (t0 // 2) * D:(t0 // 2 + 2) * D].rearrange(
                                "p (j f) -> p j f", f=D),
                            in1=psum[:].rearrange("p (j f) -> p j f", f=BANK)[
                                :, bank:bank + 2, off:off + D],
                            op=mybir.AluOpType.add,
                        ).then_inc(rt_sem, RB)
                        rb += 1
                    if not ly.batch_on_g[b]:
                        w.wait(mm_sem, BATCH * (b + 1) - RO)
                        onehot_op(vector, b, w, ohd_sem)
                while rb < ly.nrb:
                    w.wait(mm_sem, ly.retire_after_pair[rb] + 1)
                    cell0 = RB * rb
                    t0 = cell0 % CPS
                    bank = (t0 // 2) % 8
                    off = ((t0 // 16) % 8) * D
                    vector.tensor_tensor(
                        out=out_sb[:, (t0 // 2) * D:(t0 // 2 + 2) * D].rearrange(
                            "p (j f) -> p j f", f=D),
                        in0=out_sb[:, (t0 // 2) * D:(t0 // 2 + 2) * D].rearrange(
                            "p (j f) -> p j f", f=D),
                        in1=psum[:].rearrange("p (j f) -> p j f", f=BANK)[
                            :, bank:bank + 2, off:off + D],
                        op=mybir.AluOpType.add,
                    ).then_inc(rt_sem, RB)
                    rb += 1

            @block.tensor
            def _(pe):
                w = W(pe)
                for p, (c, t) in enumerate(ly.pairs):
                    cell = int(ly.pair_cell[p])
                    w.wait(c_sem, int(ly.call_of_chunk[c]) + 1)
                    b = p // BATCH
                    if ly.batch_on_g[b]:
                        w.wait(ohg_sem, int(ly.gps_done[b]))
                    else:
                        w.wait(ohd_sem, int(ly.dve_done[b]))
                    if p == ly.first_pair[cell] and ly.prev_same_slot[cell] >= 0:
                        w.wait(rt_sem, int(ly.prev_same_slot[cell]) + 1)
                    msl = int(ly.chunk_slot[c])
                    pe.matmul(
                        out=psum_cell_ap(t),
                        lhsT=oh_sb[:, (p % RO) * TILE:(p % RO + 1) * TILE],
                        rhs=msgb[:, msl * D:(msl + 1) * D],
                        start=(p == ly.first_pair[cell]),
                        stop=(p == ly.last_pair[cell]),
                        skip_group_check=True,
                    ).then_inc(mm_sem, 1)

            @block.sync
            def _(sync):
                w = W(sync)
                nq = 4
                step = NTILES // 2 // nq        # col blocks per quarter
                for qi in range(nq):
                    b0 = qi * step
                    b1 = NTILES // 2 if qi == nq - 1 else (qi + 1) * step
                    # col block b final after cell (NSEG-1)*CPS + 2b+1 retires
                    need = (NSEG - 1) * CPS + 2 * b1
                    w.wait(rt_sem, -(-need // RB) * RB)
                    sync.dma_start(
                        out=outd[:, b0 * D:b1 * D],
                        in_=out_sb[:, b0 * D:b1 * D],
                    ).then_inc(o_sem, 16)
                sync.wait_ge(o_sem, 16 * nq)

    nc.compile()
    return nc


def run_spmd(nc, per_core, trace=False):
    from concourse.bass_utils import run_bass_kernel_spmd
    return run_bass_kernel_spmd(
        nc, per_core, core_ids=list(range(len(per_core))), trace=trace
    )


def kernel(x, edge_index, _trace=False, _return_results=False):
    per_core, ly = host_prep(x, edge_index)
    nc = build_bass(ly)
    res = run_spmd(nc, per_core, trace=_trace)
    outs = []
    for k in range(NCORES):
        o = np.asarray(res.results[k]["out"])        # [128, 98*64]
        # partition p, col block j -> dest row j*128 + (p//64)*64 + (p%64)
        o = o.reshape(2, 64, NTILES // 2, D).transpose(2, 0, 1, 3).reshape(NROWS_PAD, D)
        outs.append(o[:SHARD])
    out = np.concatenate(outs, axis=0)
    if _return_results:
        return out, res
    return out
